# revision 46
# baseline (speedup 1.0000x reference)
"""AFT-Local sparse attention kernel for Trainium2, SPMD over 8 NeuronCores.

Problem (B=4, L=1024, E=256, S=32):
    Q = q @ Wq.T + bq ; K = q @ Wk.T + bk ; V = q @ Wv.T + bv
    For each (b, i, e):  per-channel softmax over the 65-wide window
        logits[j] = Q[i,e] * (K[i+j-S, e] + pb[j, e])   for |j-S| < S (strict)
        logits[j] = 0                                    for j in {0, 64} (K masked)
        logits[j] = -inf                                 for out-of-range positions
        ctx = sum_j softmax(logits)[j] * V[i+j-S, e]
    out = sigmoid(Q)^2 * ctx

Sharding: 8 cores = (batch b in 0..3) x (sequence half h in 0..1).
The h=1 half is REVERSED on the host so that every core sees an identical
problem: a sequence edge at local position 0 and valid data through the
right halo.  This keeps the SPMD graph uniform (no per-core masking).

Device layout: channels on partitions (2 halves of 128), sequence on the
free axis.  Window shifts are free AP offsets.  Per window offset d:
  DVE:  l_d = (K<<d + pb[d]) * Q           (fused scalar_tensor_tensor)
  ACT:  E_d = exp(l_d)
  DVE:  EV_d = E_d * V<<d
  PE:   N += I.T @ EV_d ; D += I.T @ E_d   (identity matmuls accumulate in PSUM)
Final: out = sigmoid(Q)^2 * N / D.

The hot path runs in bf16 (measured end-to-end error ~8e-3 vs the 2e-2
gate): halves DVE/ACT element cost and avoids the PE's fp32 HI/LO
double-pass.  K and V also exist as 1-element-shifted copies so reads at
odd window offsets stay 4-byte aligned (keeps the DVE 2x packed mode).

Raw Bass (manual semaphores): this walrus build rejects Tile's generated
sync (multi-wait instructions), so engine programs and cumulative
wait_ge thresholds are written out explicitly.
"""

import contextlib

import ml_dtypes
import numpy as np

import concourse.bass as bass
import concourse.mybir as mybir
from concourse import bass_utils

B, L, E, S = 4, 1024, 256, 32
O = 512          # output positions per core
HALO = 32        # halo on each side of the output range
NH = O + 2 * HALO  # 576: local K/V/q array length
P = 128
W = 2 * S + 1
F32 = mybir.dt.float32
BF16 = mybir.dt.bfloat16
NPBF = ml_dtypes.bfloat16

G = 7       # window offsets processed per block
NBLK = 18   # 9 blocks per channel half

TRACE = False
LAST_RESULTS = None
_DEBUG_TAP = None
_CACHE = {}

# ---- static semaphore bookkeeping ----
# sem_pe counts: 24 proj matmuls, then per eh: 4 init + 14 per block
PE_PROJ = 24


def blk_iters(b):
    return b // 9, -S + 1 + G * (b % 9)


def pe_after_block(b):
    return PE_PROJ + (4 if b < 9 else 8) + 14 * (b + 1)


PE_TOTAL = PE_PROJ + 8 + 14 * NBLK

# ACT projection ops per group (eh, t, chunk): q:1+1, k:1+1, v:2+1
PRJ_OPS = [1, 1, 1, 1, 2, 1] * 2
PRJ_CUM = np.cumsum(PRJ_OPS).tolist()          # after each group
PRJ_EH = [7, 14]                               # after each eh's projections


def _build_nc():
    nc = bass.Bass("TRN2")

    ACOLS = 2 * (NH + E)        # 1664: qT0 | Wq0 | qT1 | Wq1 per row
    BCOLS = 4 * E               # 1024: Wk0 | Wv0 | Wk1 | Wv1 per row
    FCOLS = W + 3               # 68:   pbT | biases
    blobA_d = nc.dram_tensor("blobA", [P, ACOLS], BF16, kind="ExternalInput")
    blobB_d = nc.dram_tensor("blobB", [P, BCOLS], BF16, kind="ExternalInput")
    fblob_d = [nc.dram_tensor(f"fblob{eh}", [P, FCOLS], F32,
                              kind="ExternalInput") for eh in range(2)]
    out_d = nc.dram_tensor("out", [E, O], F32, kind="ExternalOutput")

    add = mybir.AluOpType.add
    mult = mybir.AluOpType.mult
    AF = mybir.ActivationFunctionType

    ctx = contextlib.ExitStack()
    with ctx:
        sb = lambda name, shape, dt=BF16: ctx.enter_context(
            nc.sbuf_tensor(name, shape, dt))[:, :]
        ps = lambda name, shape: ctx.enter_context(
            nc.psum_tensor(name, shape, F32))[:, :]
        sem = lambda name: ctx.enter_context(nc.semaphore(name))

        blobA = sb("blobA_s", [P, 2 * (NH + E)])
        blobB = sb("blobB_s", [P, 4 * E])
        fblob = [sb(f"fblob_s{eh}", [P, W + 3], F32) for eh in range(2)]
        qT = [blobA[:, kh * (NH + E):kh * (NH + E) + NH] for kh in range(2)]
        wT = {("q", kh): blobA[:, kh * (NH + E) + NH:(kh + 1) * (NH + E)]
              for kh in range(2)}
        for kh in range(2):
            wT["k", kh] = blobB[:, 2 * kh * E:(2 * kh + 1) * E]
            wT["v", kh] = blobB[:, (2 * kh + 1) * E:(2 * kh + 2) * E]
        pb = [fblob[eh][:, 0:W] for eh in range(2)]
        bs = [fblob[eh][:, W:W + 3] for eh in range(2)]
        QKV = {(t, eh): sb(f"{t}{eh}", [P, NH])
               for t in "qkv" for eh in range(2)}
        # 1-element-shifted copies for odd window offsets (alignment)
        K1 = [sb(f"k1_{eh}", [P, NH]) for eh in range(2)]
        V1 = [sb(f"v1_{eh}", [P, NH]) for eh in range(2)]
        ident = sb("ident", [P, P])
        ones = sb("ones", [P, NH])
        tbig = [sb(f"tbig{i}", [P, G * O]) for i in range(2)]
        lbig = [sb(f"lbig{i}", [P, G * O]) for i in range(4)]
        ebig = [sb(f"ebig{i}", [P, G * O]) for i in range(4)]
        vbig = [sb(f"vbig{i}", [P, G * O]) for i in range(3)]
        epT = [sb(f"epT{eh}", [P, O], F32) for eh in range(2)]
        epLD = [sb(f"epLD{eh}", [P, O], F32) for eh in range(2)]
        epLU = [sb(f"epLU{eh}", [P, O], F32) for eh in range(2)]
        epS = [sb(f"epS{eh}", [P, O], F32) for eh in range(2)]
        ob = [sb(f"ob{eh}", [P, O], F32) for eh in range(2)]
        tapb = sb("tapb", [P, O], F32)

        prj_ps = [ps(f"prj_ps{i}", [P, O]) for i in range(2)]
        D_ps = [ps(f"D_ps{eh}", [P, O]) for eh in range(2)]
        N_ps = [ps(f"N_ps{eh}", [P, O]) for eh in range(2)]

        s_lA = sem("s_lA")  # blobA: qT + Wq
        s_lC = sem("s_lC")  # blobB: Wk + Wv
        s_lB = sem("s_lB")  # bias + pos_bias
        s_gp = sem("s_gp")
        s_prj = sem("s_prj")
        s_lg = sem("s_lg")
        s_ex = sem("s_ex")
        s_ev = sem("s_ev")
        s_pe = sem("s_pe")
        s_ea = sem("s_ea")
        s_ed = sem("s_ed")
        s_epi = sem("s_epi")
        s_od = sem("s_od")
        s_at = sem("s_at")

        def k_sh(eh, d):
            """K window-shifted AP, 4B-aligned: even offsets from K, odd
            from the 1-shifted copy."""
            o = HALO + d
            if o % 2 == 0:
                return QKV["k", eh][:, o:o + O]
            return K1[eh][:, o - 1:o - 1 + O]

        def v_sh(eh, d):
            o = HALO + d
            if o % 2 == 0:
                return QKV["v", eh][:, o:o + O]
            return V1[eh][:, o - 1:o - 1 + O]

        # projection groups: (eh, t, (n0, nn))
        groups = [(eh, t, c) for eh in range(2) for t in "qkv"
                  for c in ((0, 512), (512, NH - 512))]

        with nc.Block() as block:

            @block.sync
            def _(sync):
                # 4 consolidated loads: per-partition rows are contiguous
                # in DRAM so each DMA moves large packets. Separate
                # semaphores (HWDGE queues complete out of order).
                sync.dma_start(out=blobA, in_=blobA_d[:, :]
                               ).then_inc(s_lA, 16)
                sync.dma_start(out=blobB, in_=blobB_d[:, :]
                               ).then_inc(s_lC, 16)
                for eh in range(2):
                    sync.dma_start(out=fblob[eh], in_=fblob_d[eh][:, :]
                                   ).then_inc(s_lB, 16)
                if _DEBUG_TAP is None:
                    for eh in range(2):
                        sync.wait_ge(s_epi, eh + 1)
                        sync.dma_start(out=out_d[eh * P:(eh + 1) * P, :],
                                       in_=ob[eh]).then_inc(s_od, 16)
                    sync.wait_ge(s_od, 32)
                else:
                    sync.wait_ge(s_epi, 2)
                    tap = {
                        "D0": lambda: tapb,
                        "N0": lambda: tapb,
                        "out0": lambda: ob[0],
                    }[_DEBUG_TAP]()
                    tw = tap.shape[1]
                    sync.dma_start(out=out_d[0:P, 0:tw], in_=tap
                                   ).then_inc(s_od, 16)
                    sync.wait_ge(s_od, 16)

            @block.gpsimd
            def _(gpsimd):
                gpsimd.memset(ident, 0.0)
                gpsimd.affine_select(
                    out=ident, in_=ident,
                    compare_op=mybir.AluOpType.not_equal,
                    fill=1.0, base=0, pattern=[[-1, P]], channel_multiplier=1,
                ).then_inc(s_gp, 1)
                gpsimd.memset(ones, 1.0)
                gpsimd.memset(ones[:, 0:HALO], 0.0).then_inc(s_gp, 1)


            @block.tensor
            def _(tensor):
                tensor.wait_ge(s_gp, 2)
                # projections: ping-pong over two PSUM banks
                tensor.wait_ge(s_lA, 16)
                kv_waited = False
                for g, (eh, t, (n0, nn)) in enumerate(groups):
                    bank = prj_ps[g % 2]
                    if t != "q" and not kv_waited:
                        kv_waited = True
                        tensor.wait_ge(s_lC, 16)
                    if g >= 2:
                        tensor.wait_ge(s_prj, PRJ_CUM[g - 2])
                    for kh in range(2):
                        tensor.matmul(
                            bank[:, :nn],
                            wT[t, kh][:, eh * P:(eh + 1) * P],
                            qT[kh][:, n0:n0 + nn],
                            start=(kh == 0), stop=(kh == 1),
                        ).then_inc(s_pe, 1)
                # window accumulation, one block of G offsets at a time
                for b in range(NBLK):
                    eh, d0 = blk_iters(b)
                    if b % 9 == 0:
                        V = QKV["v", eh]
                        tensor.wait_ge(s_prj, PRJ_EH[eh])
                        tensor.matmul(D_ps[eh], ident, ones[:, HALO:HALO + O],
                                      start=True, stop=False).then_inc(s_pe, 1)
                        tensor.matmul(D_ps[eh], ident, ones[:, 0:O],
                                      start=False, stop=False).then_inc(s_pe, 1)
                        tensor.matmul(N_ps[eh], ident, V[:, 0:O],
                                      start=True, stop=False).then_inc(s_pe, 1)
                        tensor.matmul(N_ps[eh], ident,
                                      V[:, 2 * HALO:2 * HALO + O],
                                      start=False, stop=False).then_inc(s_pe, 1)
                    tensor.wait_ge(s_ev, 2 * (b + 1))
                    for g in range(G):
                        d = d0 + g
                        last = d == S - 1
                        tensor.matmul(N_ps[eh], ident,
                                      vbig[b % 3][:, g * O:(g + 1) * O],
                                      start=False, stop=last).then_inc(s_pe, 1)
                        vs = max(0, -d)
                        tensor.matmul(D_ps[eh][:, vs:], ident,
                                      ebig[b % 4][:, g * O + vs:(g + 1) * O],
                                      start=False, stop=last).then_inc(s_pe, 1)

            @block.vector
            def _(vector):
                def slab_ap(t, par, cnt, col0=0):
                    # every-other-slab view: [P, cnt, O] starting at slab par
                    return bass.AP(
                        tensor=t.tensor, offset=t.offset + (par * O) + col0,
                        ap=[t.ap[0], [2 * O, cnt], [1, O]])

                def emit_ev_block(bb):
                    ehb, d0b = blk_iters(bb)
                    vector.wait_ge(s_ex, bb + 1)
                    if bb >= 3:
                        vector.wait_ge(s_pe, pe_after_block(bb - 3))
                    for par in range(2):
                        cnt = (G + 1 - par) // 2
                        base = HALO + d0b + par
                        if base % 2 == 0:
                            vsrc, vo = QKV["v", ehb], base
                        else:
                            vsrc, vo = V1[ehb], base - 1
                        in1 = bass.AP(
                            tensor=vsrc.tensor, offset=vsrc.offset + vo,
                            ap=[vsrc.ap[0], [2, cnt], [1, O]])
                        vector.tensor_tensor(
                            out=slab_ap(vbig[bb % 3], par, cnt),
                            in0=slab_ap(ebig[bb % 4], par, cnt),
                            in1=in1, op=mult,
                        ).then_inc(s_ev, 1)

                for b in range(NBLK):
                    eh, d0 = blk_iters(b)
                    Q = QKV["q", eh]
                    if b % 9 == 0:
                        vector.wait_ge(s_prj, PRJ_EH[eh])
                        vector.tensor_copy(K1[eh][:, 0:NH - 1],
                                           QKV["k", eh][:, 1:NH])
                        vector.tensor_copy(V1[eh][:, 0:NH - 1],
                                           QKV["v", eh][:, 1:NH])
                    ndve = G if b < 2 else G - 2
                    for g in range(ndve):
                        d = d0 + g
                        vector.tensor_scalar_add(
                            tbig[b % 2][:, g * O:(g + 1) * O], k_sh(eh, d),
                            pb[eh][:, d + S:d + S + 1])
                    if b >= 2:
                        vector.wait_ge(s_at, b - 1)  # ACT-side K+pb slabs
                    if b >= 4:
                        vector.wait_ge(s_ex, b - 3)  # lbig slot free
                    qb = bass.AP(
                        tensor=Q.tensor, offset=Q.offset + HALO,
                        ap=[Q.ap[0], [0, G], [1, O]])
                    tb3 = bass.AP(
                        tensor=tbig[b % 2].tensor, offset=tbig[b % 2].offset,
                        ap=[tbig[b % 2].ap[0], [O, G], [1, O]])
                    lb3 = bass.AP(
                        tensor=lbig[b % 4].tensor, offset=lbig[b % 4].offset,
                        ap=[lbig[b % 4].ap[0], [O, G], [1, O]])
                    vector.tensor_tensor(out=lb3, in0=tb3, in1=qb, op=mult
                                         ).then_inc(s_lg, 1)
                    if b >= 2:
                        emit_ev_block(b - 2)
                    if b == 2:
                        for eh in range(2):
                            vector.wait_ge(s_ea, eh + 1)
                            vector.tensor_scalar_add(epS[eh], epT[eh], 1.0
                                                     ).then_inc(s_ed, 1)
                    if b == 13:
                        vector.wait_ge(s_ea, 5)
                        vector.scalar_tensor_tensor(
                            out=epT[0], in0=epLU[0], scalar=2.0,
                            in1=epLD[0], op0=mult, op1=add,
                        ).then_inc(s_ed, 1)
                    if b == 15:
                        vector.wait_ge(s_ea, 6)
                        vector.tensor_mul(ob[0], N_ps[0], epS[0]
                                          ).then_inc(s_epi, 1)
                for bb in (NBLK - 2, NBLK - 1):
                    emit_ev_block(bb)

                # tail: out = N * exp(-(ln D + 2 ln(1+exp(-Q))))
                if _DEBUG_TAP in ("D0", "N0"):
                    vector.wait_ge(s_pe, PE_TOTAL)
                    vector.tensor_copy(
                        tapb, D_ps[0] if _DEBUG_TAP == "D0" else N_ps[0])
                vector.wait_ge(s_ea, 7)
                vector.scalar_tensor_tensor(
                    out=epT[1], in0=epLU[1], scalar=2.0,
                    in1=epLD[1], op0=mult, op1=add,
                ).then_inc(s_ed, 1)
                vector.wait_ge(s_ea, 8)
                vector.tensor_mul(ob[1], N_ps[1], epS[1]
                                  ).then_inc(s_epi, 1)

            @block.scalar
            def _(scalar):
                # projections: add bias, move PSUM -> SBUF
                scalar.wait_ge(s_lB, 32)  # bias + pos_bias present
                for g, (eh, t, (n0, nn)) in enumerate(groups):
                    ti = "qkv".index(t)
                    bank = prj_ps[g % 2]
                    scalar.wait_ge(s_pe, 2 * (g + 1))
                    T_sb = QKV[t, eh]
                    if t == "v" and n0 == 0:
                        scalar.activation(T_sb[:, 0:HALO], bank[:, 0:HALO],
                                          AF.Copy).then_inc(s_prj, 1)
                        scalar.activation(
                            T_sb[:, HALO:nn], bank[:, HALO:nn], AF.Identity,
                            bias=bs[eh][:, ti:ti + 1], scale=1.0,
                        ).then_inc(s_prj, 1)
                    else:
                        scalar.activation(
                            T_sb[:, n0:n0 + nn], bank[:, :nn], AF.Identity,
                            bias=bs[eh][:, ti:ti + 1], scale=1.0,
                        ).then_inc(s_prj, 1)
                for eh in range(2):
                    scalar.activation(epT[eh], QKV["q", eh][:, HALO:HALO + O],
                                      AF.Exp, scale=-1.0).then_inc(s_ea, 1)
                for b in range(NBLK):
                    ehb, d0b = blk_iters(b)
                    if b >= 2:
                        scalar.wait_ge(s_lg, b - 1)  # tbig slot free
                        for g in (G - 2, G - 1):
                            d = d0b + g
                            o = HALO + d
                            ai = scalar.activation(
                                tbig[b % 2][:, g * O:(g + 1) * O],
                                QKV["k", ehb][:, o:o + O], AF.Identity,
                                bias=pb[ehb][:, d + S:d + S + 1], scale=1.0)
                        ai.then_inc(s_at, 1)
                    if b >= 1:
                        bb = b - 1
                        scalar.wait_ge(s_lg, bb + 1)
                        if bb >= 4:
                            scalar.wait_ge(s_pe, pe_after_block(bb - 4))
                        scalar.activation(ebig[bb % 4], lbig[bb % 4], AF.Exp
                                          ).then_inc(s_ex, 1)
                        if bb == 4:
                            # LU = ln(1+exp(-Q)), both eh (u ready early)
                            for eh in range(2):
                                scalar.wait_ge(s_ed, eh + 1)
                                scalar.activation(epLU[eh], epS[eh], AF.Ln
                                                  ).then_inc(s_ea, 1)
                        if bb == 10:
                            scalar.wait_ge(s_pe, pe_after_block(8))
                            scalar.activation(epLD[0], D_ps[0], AF.Ln
                                              ).then_inc(s_ea, 1)
                        if bb == 12:
                            scalar.wait_ge(s_ed, 3)
                            scalar.activation(epS[0], epT[0], AF.Exp,
                                              scale=-1.0).then_inc(s_ea, 1)
                bb = NBLK - 1
                scalar.wait_ge(s_lg, bb + 1)
                scalar.wait_ge(s_pe, pe_after_block(bb - 4))
                scalar.activation(ebig[bb % 4], lbig[bb % 4], AF.Exp
                                  ).then_inc(s_ex, 1)
                # epilogue, same exp/ln table set (no set switch).
                # sigma-side (T = exp(-Q), LU = ln(1+T)) runs early; only
                # the D-dependent part trails the PE.
                scalar.wait_ge(s_pe, PE_TOTAL)
                scalar.activation(epLD[1], D_ps[1], AF.Ln).then_inc(s_ea, 1)
                scalar.wait_ge(s_ed, 4)
                scalar.activation(epS[1], epT[1], AF.Exp, scale=-1.0
                                  ).then_inc(s_ea, 1)

    return nc


def _shard_inputs(q, Wq, bq, Wk, bk, Wv, bv, pos_bias):
    """Build per-core input maps. Core c = 2*b + h."""
    wqT = Wq.T.astype(NPBF)
    wkT = Wk.T.astype(NPBF)
    wvT = Wv.T.astype(NPBF)
    bias = np.stack([bq, bk, bv], axis=1).astype(np.float32)   # [E, 3]
    pbT_f = pos_bias.T.astype(np.float32)                      # [E, W]
    pbT_r = pos_bias[::-1].T.astype(np.float32)                # reversed

    blobB = np.ascontiguousarray(np.concatenate(
        [wkT[0:P], wvT[0:P], wkT[P:E], wvT[P:E]], axis=1))     # [P, 4E]
    in_maps = []
    for c in range(8):
        b, h = divmod(c, 2)
        qh = np.zeros((NH, E), np.float32)
        if h == 0:
            qh[HALO:] = q[b, 0:O + HALO]          # positions -32..543, pad<0
        else:
            qh[HALO:] = q[b, L - (O + HALO):][::-1]  # reversed right half
        qT = qh.T.astype(NPBF)                                 # [E, NH]
        pbT = pbT_f if h == 0 else pbT_r
        fb = np.concatenate([pbT, bias], axis=1)               # [E, W+3]
        in_maps.append({
            "blobA": np.ascontiguousarray(np.concatenate(
                [qT[0:P], wqT[0:P], qT[P:E], wqT[P:E]], axis=1)),
            "blobB": blobB,
            "fblob0": np.ascontiguousarray(fb[0:P]),
            "fblob1": np.ascontiguousarray(fb[P:E]),
        })
    return in_maps


def _unshard(results):
    out = np.empty((B, L, E), np.float32)
    for c in range(8):
        b, h = divmod(c, 2)
        o_core = np.asarray(results[c]["out"], np.float32).T  # [O, E]
        if h == 0:
            out[b, 0:O] = o_core
        else:
            out[b, L - O:] = o_core[::-1]
    return out


def kernel(q, Wq, bq, Wk, bk, Wv, bv, pos_bias):
    global LAST_RESULTS
    q = np.asarray(q, np.float32)
    if "nc" not in _CACHE:
        _CACHE["nc"] = _build_nc()
    nc = _CACHE["nc"]
    in_maps = _shard_inputs(q, np.asarray(Wq), np.asarray(bq), np.asarray(Wk),
                            np.asarray(bk), np.asarray(Wv), np.asarray(bv),
                            np.asarray(pos_bias))
    res = bass_utils.run_bass_kernel_spmd(
        nc, in_maps, core_ids=list(range(8)), trace=TRACE,
    )
    LAST_RESULTS = res
    return _unshard(res.results)


# revision 47
# speedup vs baseline: 1.0054x; 1.0054x over previous
"""AFT-Local sparse attention kernel for Trainium2, SPMD over 8 NeuronCores.

Problem (B=4, L=1024, E=256, S=32):
    Q = q @ Wq.T + bq ; K = q @ Wk.T + bk ; V = q @ Wv.T + bv
    For each (b, i, e):  per-channel softmax over the 65-wide window
        logits[j] = Q[i,e] * (K[i+j-S, e] + pb[j, e])   for |j-S| < S (strict)
        logits[j] = 0                                    for j in {0, 64} (K masked)
        logits[j] = -inf                                 for out-of-range positions
        ctx = sum_j softmax(logits)[j] * V[i+j-S, e]
    out = sigmoid(Q)^2 * ctx

Sharding: 8 cores = (batch b in 0..3) x (sequence half h in 0..1).
The h=1 half is REVERSED on the host so that every core sees an identical
problem: a sequence edge at local position 0 and valid data through the
right halo.  This keeps the SPMD graph uniform (no per-core masking).

Device layout: channels on partitions (2 halves of 128), sequence on the
free axis.  Window shifts are free AP offsets.  Per window offset d:
  DVE:  l_d = (K<<d + pb[d]) * Q           (fused scalar_tensor_tensor)
  ACT:  E_d = exp(l_d)
  DVE:  EV_d = E_d * V<<d
  PE:   N += I.T @ EV_d ; D += I.T @ E_d   (identity matmuls accumulate in PSUM)
Final: out = sigmoid(Q)^2 * N / D.

The hot path runs in bf16 (measured end-to-end error ~8e-3 vs the 2e-2
gate): halves DVE/ACT element cost and avoids the PE's fp32 HI/LO
double-pass.  K and V also exist as 1-element-shifted copies so reads at
odd window offsets stay 4-byte aligned (keeps the DVE 2x packed mode).

Raw Bass (manual semaphores): this walrus build rejects Tile's generated
sync (multi-wait instructions), so engine programs and cumulative
wait_ge thresholds are written out explicitly.
"""

import contextlib

import ml_dtypes
import numpy as np

import concourse.bass as bass
import concourse.mybir as mybir
from concourse import bass_utils

B, L, E, S = 4, 1024, 256, 32
O = 512          # output positions per core
HALO = 32        # halo on each side of the output range
NH = O + 2 * HALO  # 576: local K/V/q array length
P = 128
W = 2 * S + 1
F32 = mybir.dt.float32
BF16 = mybir.dt.bfloat16
NPBF = ml_dtypes.bfloat16

G = 7       # window offsets processed per block
NBLK = 18   # 9 blocks per channel half

TRACE = False
LAST_RESULTS = None
_DEBUG_TAP = None
_CACHE = {}

# ---- static semaphore bookkeeping ----
# sem_pe counts: 24 proj matmuls, then per eh: 4 init + 14 per block
PE_PROJ = 24


def blk_iters(b):
    return b // 9, -S + 1 + G * (b % 9)


def pe_after_block(b):
    return PE_PROJ + (4 if b < 9 else 8) + 14 * (b + 1)


PE_TOTAL = PE_PROJ + 8 + 14 * NBLK

# Projection copy-outs: groups 0-3 (eh0 q,k) on the DVE (s_pd, 1/group);
# groups 4-11 on the ScalarE (s_prj; v-chunk0 groups emit 2 ops).
ACT_CUM = {4: 2, 5: 3, 6: 4, 7: 5, 8: 6, 9: 7, 10: 9, 11: 10}


def _build_nc():
    nc = bass.Bass("TRN2")

    ACOLS = 2 * (NH + E)        # 1664: qT0 | Wq0 | qT1 | Wq1 per row
    BCOLS = 4 * E               # 1024: Wk0 | Wv0 | Wk1 | Wv1 per row
    FCOLS = W + 3               # 68:   pbT | biases
    blobA_d = nc.dram_tensor("blobA", [P, ACOLS], BF16, kind="ExternalInput")
    blobB_d = nc.dram_tensor("blobB", [P, BCOLS], BF16, kind="ExternalInput")
    fblob_d = [nc.dram_tensor(f"fblob{eh}", [P, FCOLS], F32,
                              kind="ExternalInput") for eh in range(2)]
    out_d = nc.dram_tensor("out", [E, O], F32, kind="ExternalOutput")

    add = mybir.AluOpType.add
    mult = mybir.AluOpType.mult
    AF = mybir.ActivationFunctionType

    ctx = contextlib.ExitStack()
    with ctx:
        sb = lambda name, shape, dt=BF16: ctx.enter_context(
            nc.sbuf_tensor(name, shape, dt))[:, :]
        ps = lambda name, shape: ctx.enter_context(
            nc.psum_tensor(name, shape, F32))[:, :]
        sem = lambda name: ctx.enter_context(nc.semaphore(name))

        blobA = sb("blobA_s", [P, 2 * (NH + E)])
        blobB = sb("blobB_s", [P, 4 * E])
        fblob = [sb(f"fblob_s{eh}", [P, W + 3], F32) for eh in range(2)]
        qT = [blobA[:, kh * (NH + E):kh * (NH + E) + NH] for kh in range(2)]
        wT = {("q", kh): blobA[:, kh * (NH + E) + NH:(kh + 1) * (NH + E)]
              for kh in range(2)}
        for kh in range(2):
            wT["k", kh] = blobB[:, 2 * kh * E:(2 * kh + 1) * E]
            wT["v", kh] = blobB[:, (2 * kh + 1) * E:(2 * kh + 2) * E]
        pb = [fblob[eh][:, 0:W] for eh in range(2)]
        bs = [fblob[eh][:, W:W + 3] for eh in range(2)]
        QKV = {(t, eh): sb(f"{t}{eh}", [P, NH])
               for t in "qkv" for eh in range(2)}
        # 1-element-shifted copies for odd window offsets (alignment)
        K1 = [sb(f"k1_{eh}", [P, NH]) for eh in range(2)]
        V1 = [sb(f"v1_{eh}", [P, NH]) for eh in range(2)]
        ident = sb("ident", [P, P])
        ones = sb("ones", [P, NH])
        tbig = [sb(f"tbig{i}", [P, G * O]) for i in range(2)]
        lbig = [sb(f"lbig{i}", [P, G * O]) for i in range(4)]
        ebig = [sb(f"ebig{i}", [P, G * O]) for i in range(4)]
        vbig = [sb(f"vbig{i}", [P, G * O]) for i in range(3)]
        epT = [sb(f"epT{eh}", [P, O], F32) for eh in range(2)]
        epLD = [sb(f"epLD{eh}", [P, O], F32) for eh in range(2)]
        epLU = [sb(f"epLU{eh}", [P, O], F32) for eh in range(2)]
        epS = [sb(f"epS{eh}", [P, O], F32) for eh in range(2)]
        ob = [sb(f"ob{eh}", [P, O], F32) for eh in range(2)]
        tapb = sb("tapb", [P, O], F32)

        prj_ps = [ps(f"prj_ps{i}", [P, O]) for i in range(2)]
        D_ps = [ps(f"D_ps{eh}", [P, O]) for eh in range(2)]
        N_ps = [ps(f"N_ps{eh}", [P, O]) for eh in range(2)]

        s_lA = sem("s_lA")  # blobA: qT + Wq
        s_lC = sem("s_lC")  # blobB: Wk + Wv
        s_lB = sem("s_lB")  # bias + pos_bias
        s_gp = sem("s_gp")
        s_prj = sem("s_prj")
        s_pd = sem("s_pd")
        s_lg = sem("s_lg")
        s_ex = sem("s_ex")
        s_ev = sem("s_ev")
        s_pe = sem("s_pe")
        s_ea = sem("s_ea")
        s_ed = sem("s_ed")
        s_epi = sem("s_epi")
        s_od = sem("s_od")
        s_at = sem("s_at")

        def k_sh(eh, d):
            """K window-shifted AP, 4B-aligned: even offsets from K, odd
            from the 1-shifted copy."""
            o = HALO + d
            if o % 2 == 0:
                return QKV["k", eh][:, o:o + O]
            return K1[eh][:, o - 1:o - 1 + O]

        def v_sh(eh, d):
            o = HALO + d
            if o % 2 == 0:
                return QKV["v", eh][:, o:o + O]
            return V1[eh][:, o - 1:o - 1 + O]

        # projection groups: (eh, t, (n0, nn))
        groups = [(eh, t, c) for eh in range(2) for t in "qkv"
                  for c in ((0, 512), (512, NH - 512))]

        with nc.Block() as block:

            @block.sync
            def _(sync):
                # 4 consolidated loads: per-partition rows are contiguous
                # in DRAM so each DMA moves large packets. Separate
                # semaphores (HWDGE queues complete out of order).
                sync.dma_start(out=blobA, in_=blobA_d[:, :]
                               ).then_inc(s_lA, 16)
                sync.dma_start(out=blobB, in_=blobB_d[:, :]
                               ).then_inc(s_lC, 16)
                for eh in range(2):
                    sync.dma_start(out=fblob[eh], in_=fblob_d[eh][:, :]
                                   ).then_inc(s_lB, 16)
                if _DEBUG_TAP is None:
                    for eh in range(2):
                        sync.wait_ge(s_epi, eh + 1)
                        sync.dma_start(out=out_d[eh * P:(eh + 1) * P, :],
                                       in_=ob[eh]).then_inc(s_od, 16)
                    sync.wait_ge(s_od, 32)
                else:
                    sync.wait_ge(s_epi, 2)
                    tap = {
                        "D0": lambda: tapb,
                        "N0": lambda: tapb,
                        "out0": lambda: ob[0],
                    }[_DEBUG_TAP]()
                    tw = tap.shape[1]
                    sync.dma_start(out=out_d[0:P, 0:tw], in_=tap
                                   ).then_inc(s_od, 16)
                    sync.wait_ge(s_od, 16)

            @block.gpsimd
            def _(gpsimd):
                gpsimd.memset(ident, 0.0)
                gpsimd.affine_select(
                    out=ident, in_=ident,
                    compare_op=mybir.AluOpType.not_equal,
                    fill=1.0, base=0, pattern=[[-1, P]], channel_multiplier=1,
                ).then_inc(s_gp, 1)
                gpsimd.memset(ones, 1.0)
                gpsimd.memset(ones[:, 0:HALO], 0.0).then_inc(s_gp, 1)


            @block.tensor
            def _(tensor):
                tensor.wait_ge(s_gp, 2)
                # projections: ping-pong over two PSUM banks
                tensor.wait_ge(s_lA, 16)
                kv_waited = False
                for g, (eh, t, (n0, nn)) in enumerate(groups):
                    bank = prj_ps[g % 2]
                    if t != "q" and not kv_waited:
                        kv_waited = True
                        tensor.wait_ge(s_lC, 16)
                    if g >= 2:
                        dep = g - 2
                        if dep <= 3:
                            tensor.wait_ge(s_pd, dep + 1)
                        else:
                            tensor.wait_ge(s_prj, ACT_CUM[dep])
                    for kh in range(2):
                        tensor.matmul(
                            bank[:, :nn],
                            wT[t, kh][:, eh * P:(eh + 1) * P],
                            qT[kh][:, n0:n0 + nn],
                            start=(kh == 0), stop=(kh == 1),
                        ).then_inc(s_pe, 1)
                # window accumulation, one block of G offsets at a time
                for b in range(NBLK):
                    eh, d0 = blk_iters(b)
                    if b % 9 == 0:
                        V = QKV["v", eh]
                        tensor.wait_ge(s_prj, 3 if eh == 0 else 10)
                        tensor.matmul(D_ps[eh], ident, ones[:, HALO:HALO + O],
                                      start=True, stop=False).then_inc(s_pe, 1)
                        tensor.matmul(D_ps[eh], ident, ones[:, 0:O],
                                      start=False, stop=False).then_inc(s_pe, 1)
                        tensor.matmul(N_ps[eh], ident, V[:, 0:O],
                                      start=True, stop=False).then_inc(s_pe, 1)
                        tensor.matmul(N_ps[eh], ident,
                                      V[:, 2 * HALO:2 * HALO + O],
                                      start=False, stop=False).then_inc(s_pe, 1)
                    tensor.wait_ge(s_ev, 2 * (b + 1))
                    for g in range(G):
                        d = d0 + g
                        last = d == S - 1
                        tensor.matmul(N_ps[eh], ident,
                                      vbig[b % 3][:, g * O:(g + 1) * O],
                                      start=False, stop=last).then_inc(s_pe, 1)
                        vs = max(0, -d)
                        tensor.matmul(D_ps[eh][:, vs:], ident,
                                      ebig[b % 4][:, g * O + vs:(g + 1) * O],
                                      start=False, stop=last).then_inc(s_pe, 1)

            @block.vector
            def _(vector):
                def slab_ap(t, par, cnt, col0=0):
                    # every-other-slab view: [P, cnt, O] starting at slab par
                    return bass.AP(
                        tensor=t.tensor, offset=t.offset + (par * O) + col0,
                        ap=[t.ap[0], [2 * O, cnt], [1, O]])

                def emit_ev_block(bb):
                    ehb, d0b = blk_iters(bb)
                    vector.wait_ge(s_ex, bb + 1)
                    if bb >= 3:
                        vector.wait_ge(s_pe, pe_after_block(bb - 3))
                    for par in range(2):
                        cnt = (G + 1 - par) // 2
                        base = HALO + d0b + par
                        if base % 2 == 0:
                            vsrc, vo = QKV["v", ehb], base
                        else:
                            vsrc, vo = V1[ehb], base - 1
                        in1 = bass.AP(
                            tensor=vsrc.tensor, offset=vsrc.offset + vo,
                            ap=[vsrc.ap[0], [2, cnt], [1, O]])
                        vector.tensor_tensor(
                            out=slab_ap(vbig[bb % 3], par, cnt),
                            in0=slab_ap(ebig[bb % 4], par, cnt),
                            in1=in1, op=mult,
                        ).then_inc(s_ev, 1)

                vector.wait_ge(s_lB, 32)  # bias scalars present
                for g in range(4):
                    ehg, tg, (n0, nn) = groups[g]
                    ti = "qkv".index(tg)
                    vector.wait_ge(s_pe, 2 * (g + 1))
                    vector.tensor_scalar_add(
                        QKV[tg, ehg][:, n0:n0 + nn], prj_ps[g % 2][:, :nn],
                        bs[ehg][:, ti:ti + 1]).then_inc(s_pd, 1)
                for b in range(NBLK):
                    eh, d0 = blk_iters(b)
                    Q = QKV["q", eh]
                    if b % 9 == 0:
                        vector.wait_ge(s_prj, 3 if eh == 0 else 10)
                        vector.tensor_copy(K1[eh][:, 0:NH - 1],
                                           QKV["k", eh][:, 1:NH])
                        vector.tensor_copy(V1[eh][:, 0:NH - 1],
                                           QKV["v", eh][:, 1:NH])
                    ndve = G if b < 2 else G - 2
                    for g in range(ndve):
                        d = d0 + g
                        vector.tensor_scalar_add(
                            tbig[b % 2][:, g * O:(g + 1) * O], k_sh(eh, d),
                            pb[eh][:, d + S:d + S + 1])
                    if b >= 2:
                        vector.wait_ge(s_at, b - 1)  # ACT-side K+pb slabs
                    if b >= 4:
                        vector.wait_ge(s_ex, b - 3)  # lbig slot free
                    qb = bass.AP(
                        tensor=Q.tensor, offset=Q.offset + HALO,
                        ap=[Q.ap[0], [0, G], [1, O]])
                    tb3 = bass.AP(
                        tensor=tbig[b % 2].tensor, offset=tbig[b % 2].offset,
                        ap=[tbig[b % 2].ap[0], [O, G], [1, O]])
                    lb3 = bass.AP(
                        tensor=lbig[b % 4].tensor, offset=lbig[b % 4].offset,
                        ap=[lbig[b % 4].ap[0], [O, G], [1, O]])
                    vector.tensor_tensor(out=lb3, in0=tb3, in1=qb, op=mult
                                         ).then_inc(s_lg, 1)
                    if b >= 2:
                        emit_ev_block(b - 2)
                    if b == 2:
                        for eh in range(2):
                            vector.wait_ge(s_ea, eh + 1)
                            vector.tensor_scalar_add(epS[eh], epT[eh], 1.0
                                                     ).then_inc(s_ed, 1)
                    if b == 13:
                        vector.wait_ge(s_ea, 5)
                        vector.scalar_tensor_tensor(
                            out=epT[0], in0=epLU[0], scalar=2.0,
                            in1=epLD[0], op0=mult, op1=add,
                        ).then_inc(s_ed, 1)
                    if b == 15:
                        vector.wait_ge(s_ea, 6)
                        vector.tensor_mul(ob[0], N_ps[0], epS[0]
                                          ).then_inc(s_epi, 1)
                for bb in (NBLK - 2, NBLK - 1):
                    emit_ev_block(bb)

                # tail: out = N * exp(-(ln D + 2 ln(1+exp(-Q))))
                if _DEBUG_TAP in ("D0", "N0"):
                    vector.wait_ge(s_pe, PE_TOTAL)
                    vector.tensor_copy(
                        tapb, D_ps[0] if _DEBUG_TAP == "D0" else N_ps[0])
                vector.wait_ge(s_ea, 7)
                vector.scalar_tensor_tensor(
                    out=epT[1], in0=epLU[1], scalar=2.0,
                    in1=epLD[1], op0=mult, op1=add,
                ).then_inc(s_ed, 1)
                vector.wait_ge(s_ea, 8)
                vector.tensor_mul(ob[1], N_ps[1], epS[1]
                                  ).then_inc(s_epi, 1)

            @block.scalar
            def _(scalar):
                # projections: add bias, move PSUM -> SBUF (groups 4-11)
                scalar.wait_ge(s_lB, 32)  # bias + pos_bias present
                for g in range(4, 12):
                    eh, t, (n0, nn) = groups[g]
                    ti = "qkv".index(t)
                    bank = prj_ps[g % 2]
                    scalar.wait_ge(s_pe, 2 * (g + 1))
                    T_sb = QKV[t, eh]
                    if t == "v" and n0 == 0:
                        scalar.activation(T_sb[:, 0:HALO], bank[:, 0:HALO],
                                          AF.Copy).then_inc(s_prj, 1)
                        scalar.activation(
                            T_sb[:, HALO:nn], bank[:, HALO:nn], AF.Identity,
                            bias=bs[eh][:, ti:ti + 1], scale=1.0,
                        ).then_inc(s_prj, 1)
                    else:
                        scalar.activation(
                            T_sb[:, n0:n0 + nn], bank[:, :nn], AF.Identity,
                            bias=bs[eh][:, ti:ti + 1], scale=1.0,
                        ).then_inc(s_prj, 1)
                scalar.wait_ge(s_pd, 2)  # eh0 Q from the DVE
                for eh in range(2):
                    scalar.activation(epT[eh], QKV["q", eh][:, HALO:HALO + O],
                                      AF.Exp, scale=-1.0).then_inc(s_ea, 1)
                for b in range(NBLK):
                    ehb, d0b = blk_iters(b)
                    if b >= 2:
                        if b == 2:
                            scalar.wait_ge(s_pd, 4)  # eh0 K from the DVE
                        scalar.wait_ge(s_lg, b - 1)  # tbig slot free
                        for g in (G - 2, G - 1):
                            d = d0b + g
                            o = HALO + d
                            ai = scalar.activation(
                                tbig[b % 2][:, g * O:(g + 1) * O],
                                QKV["k", ehb][:, o:o + O], AF.Identity,
                                bias=pb[ehb][:, d + S:d + S + 1], scale=1.0)
                        ai.then_inc(s_at, 1)
                    if b >= 1:
                        bb = b - 1
                        scalar.wait_ge(s_lg, bb + 1)
                        if bb >= 4:
                            scalar.wait_ge(s_pe, pe_after_block(bb - 4))
                        scalar.activation(ebig[bb % 4], lbig[bb % 4], AF.Exp
                                          ).then_inc(s_ex, 1)
                        if bb == 4:
                            # LU = ln(1+exp(-Q)), both eh (u ready early)
                            for eh in range(2):
                                scalar.wait_ge(s_ed, eh + 1)
                                scalar.activation(epLU[eh], epS[eh], AF.Ln
                                                  ).then_inc(s_ea, 1)
                        if bb == 10:
                            scalar.wait_ge(s_pe, pe_after_block(8))
                            scalar.activation(epLD[0], D_ps[0], AF.Ln
                                              ).then_inc(s_ea, 1)
                        if bb == 12:
                            scalar.wait_ge(s_ed, 3)
                            scalar.activation(epS[0], epT[0], AF.Exp,
                                              scale=-1.0).then_inc(s_ea, 1)
                bb = NBLK - 1
                scalar.wait_ge(s_lg, bb + 1)
                scalar.wait_ge(s_pe, pe_after_block(bb - 4))
                scalar.activation(ebig[bb % 4], lbig[bb % 4], AF.Exp
                                  ).then_inc(s_ex, 1)
                # epilogue, same exp/ln table set (no set switch).
                # sigma-side (T = exp(-Q), LU = ln(1+T)) runs early; only
                # the D-dependent part trails the PE.
                scalar.wait_ge(s_pe, PE_TOTAL)
                scalar.activation(epLD[1], D_ps[1], AF.Ln).then_inc(s_ea, 1)
                scalar.wait_ge(s_ed, 4)
                scalar.activation(epS[1], epT[1], AF.Exp, scale=-1.0
                                  ).then_inc(s_ea, 1)

    return nc


def _shard_inputs(q, Wq, bq, Wk, bk, Wv, bv, pos_bias):
    """Build per-core input maps. Core c = 2*b + h."""
    wqT = Wq.T.astype(NPBF)
    wkT = Wk.T.astype(NPBF)
    wvT = Wv.T.astype(NPBF)
    bias = np.stack([bq, bk, bv], axis=1).astype(np.float32)   # [E, 3]
    pbT_f = pos_bias.T.astype(np.float32)                      # [E, W]
    pbT_r = pos_bias[::-1].T.astype(np.float32)                # reversed

    blobB = np.ascontiguousarray(np.concatenate(
        [wkT[0:P], wvT[0:P], wkT[P:E], wvT[P:E]], axis=1))     # [P, 4E]
    in_maps = []
    for c in range(8):
        b, h = divmod(c, 2)
        qh = np.zeros((NH, E), np.float32)
        if h == 0:
            qh[HALO:] = q[b, 0:O + HALO]          # positions -32..543, pad<0
        else:
            qh[HALO:] = q[b, L - (O + HALO):][::-1]  # reversed right half
        qT = qh.T.astype(NPBF)                                 # [E, NH]
        pbT = pbT_f if h == 0 else pbT_r
        fb = np.concatenate([pbT, bias], axis=1)               # [E, W+3]
        in_maps.append({
            "blobA": np.ascontiguousarray(np.concatenate(
                [qT[0:P], wqT[0:P], qT[P:E], wqT[P:E]], axis=1)),
            "blobB": blobB,
            "fblob0": np.ascontiguousarray(fb[0:P]),
            "fblob1": np.ascontiguousarray(fb[P:E]),
        })
    return in_maps


def _unshard(results):
    out = np.empty((B, L, E), np.float32)
    for c in range(8):
        b, h = divmod(c, 2)
        o_core = np.asarray(results[c]["out"], np.float32).T  # [O, E]
        if h == 0:
            out[b, 0:O] = o_core
        else:
            out[b, L - O:] = o_core[::-1]
    return out


def kernel(q, Wq, bq, Wk, bk, Wv, bv, pos_bias):
    global LAST_RESULTS
    q = np.asarray(q, np.float32)
    if "nc" not in _CACHE:
        _CACHE["nc"] = _build_nc()
    nc = _CACHE["nc"]
    in_maps = _shard_inputs(q, np.asarray(Wq), np.asarray(bq), np.asarray(Wk),
                            np.asarray(bk), np.asarray(Wv), np.asarray(bv),
                            np.asarray(pos_bias))
    res = bass_utils.run_bass_kernel_spmd(
        nc, in_maps, core_ids=list(range(8)), trace=TRACE,
    )
    LAST_RESULTS = res
    return _unshard(res.results)


# revision 48
# speedup vs baseline: 1.0069x; 1.0015x over previous
"""AFT-Local sparse attention kernel for Trainium2, SPMD over 8 NeuronCores.

Problem (B=4, L=1024, E=256, S=32):
    Q = q @ Wq.T + bq ; K = q @ Wk.T + bk ; V = q @ Wv.T + bv
    For each (b, i, e):  per-channel softmax over the 65-wide window
        logits[j] = Q[i,e] * (K[i+j-S, e] + pb[j, e])   for |j-S| < S (strict)
        logits[j] = 0                                    for j in {0, 64} (K masked)
        logits[j] = -inf                                 for out-of-range positions
        ctx = sum_j softmax(logits)[j] * V[i+j-S, e]
    out = sigmoid(Q)^2 * ctx

Sharding: 8 cores = (batch b in 0..3) x (sequence half h in 0..1).
The h=1 half is REVERSED on the host so that every core sees an identical
problem: a sequence edge at local position 0 and valid data through the
right halo.  This keeps the SPMD graph uniform (no per-core masking).

Device layout: channels on partitions (2 halves of 128), sequence on the
free axis.  Window shifts are free AP offsets.  Per window offset d:
  DVE:  l_d = (K<<d + pb[d]) * Q           (fused scalar_tensor_tensor)
  ACT:  E_d = exp(l_d)
  DVE:  EV_d = E_d * V<<d
  PE:   N += I.T @ EV_d ; D += I.T @ E_d   (identity matmuls accumulate in PSUM)
Final: out = sigmoid(Q)^2 * N / D.

The hot path runs in bf16 (measured end-to-end error ~8e-3 vs the 2e-2
gate): halves DVE/ACT element cost and avoids the PE's fp32 HI/LO
double-pass.  K and V also exist as 1-element-shifted copies so reads at
odd window offsets stay 4-byte aligned (keeps the DVE 2x packed mode).

Raw Bass (manual semaphores): this walrus build rejects Tile's generated
sync (multi-wait instructions), so engine programs and cumulative
wait_ge thresholds are written out explicitly.
"""

import contextlib

import ml_dtypes
import numpy as np

import concourse.bass as bass
import concourse.mybir as mybir
from concourse import bass_utils

B, L, E, S = 4, 1024, 256, 32
O = 512          # output positions per core
HALO = 32        # halo on each side of the output range
NH = O + 2 * HALO  # 576: local K/V/q array length
P = 128
W = 2 * S + 1
F32 = mybir.dt.float32
BF16 = mybir.dt.bfloat16
NPBF = ml_dtypes.bfloat16

G = 7       # window offsets processed per block
NBLK = 18   # 9 blocks per channel half

TRACE = False
LAST_RESULTS = None
_DEBUG_TAP = None
_CACHE = {}

# ---- static semaphore bookkeeping ----
# sem_pe counts: 24 proj matmuls, then per eh: 4 init + 14 per block
PE_PROJ = 24


def blk_iters(b):
    return b // 9, -S + 1 + G * (b % 9)


def pe_after_block(b):
    return PE_PROJ + (4 if b < 9 else 8) + 14 * (b + 1)


PE_TOTAL = PE_PROJ + 8 + 14 * NBLK

# Projection copy-outs: groups 0-3 (eh0 q,k) on the DVE (s_pd, 1/group);
# groups 4-11 on the ScalarE (s_prj; v-chunk0 groups emit 2 ops).
ACT_CUM = {4: 2, 5: 3, 6: 4, 7: 5, 8: 6, 9: 7, 10: 9, 11: 10}


def _build_nc():
    nc = bass.Bass("TRN2")

    ACOLS = 2 * (NH + E)        # 1664: qT0 | Wq0 | qT1 | Wq1 per row
    BCOLS = 4 * E               # 1024: Wk0 | Wv0 | Wk1 | Wv1 per row
    FCOLS = W + 3               # 68:   pbT | biases
    blobA_d = nc.dram_tensor("blobA", [P, ACOLS], BF16, kind="ExternalInput")
    blobB_d = nc.dram_tensor("blobB", [P, BCOLS], BF16, kind="ExternalInput")
    fblob_d = [nc.dram_tensor(f"fblob{eh}", [P, FCOLS], F32,
                              kind="ExternalInput") for eh in range(2)]
    out_d = nc.dram_tensor("out", [E, O], F32, kind="ExternalOutput")

    add = mybir.AluOpType.add
    mult = mybir.AluOpType.mult
    AF = mybir.ActivationFunctionType

    ctx = contextlib.ExitStack()
    with ctx:
        sb = lambda name, shape, dt=BF16: ctx.enter_context(
            nc.sbuf_tensor(name, shape, dt))[:, :]
        ps = lambda name, shape: ctx.enter_context(
            nc.psum_tensor(name, shape, F32))[:, :]
        sem = lambda name: ctx.enter_context(nc.semaphore(name))

        blobA = sb("blobA_s", [P, 2 * (NH + E)])
        blobB = sb("blobB_s", [P, 4 * E])
        fblob = [sb(f"fblob_s{eh}", [P, W + 3], F32) for eh in range(2)]
        qT = [blobA[:, kh * (NH + E):kh * (NH + E) + NH] for kh in range(2)]
        wT = {("q", kh): blobA[:, kh * (NH + E) + NH:(kh + 1) * (NH + E)]
              for kh in range(2)}
        for kh in range(2):
            wT["k", kh] = blobB[:, 2 * kh * E:(2 * kh + 1) * E]
            wT["v", kh] = blobB[:, (2 * kh + 1) * E:(2 * kh + 2) * E]
        pb = [fblob[eh][:, 0:W] for eh in range(2)]
        bs = [fblob[eh][:, W:W + 3] for eh in range(2)]
        QKV = {(t, eh): sb(f"{t}{eh}", [P, NH])
               for t in "qkv" for eh in range(2)}
        # 1-element-shifted copies for odd window offsets (alignment)
        K1 = [sb(f"k1_{eh}", [P, NH]) for eh in range(2)]
        V1 = [sb(f"v1_{eh}", [P, NH]) for eh in range(2)]
        ident = sb("ident", [P, P])
        ones = sb("ones", [P, NH])
        tbig = [sb(f"tbig{i}", [P, G * O]) for i in range(2)]
        lbig = [sb(f"lbig{i}", [P, G * O]) for i in range(4)]
        ebig = [sb(f"ebig{i}", [P, G * O]) for i in range(4)]
        vbig = [sb(f"vbig{i}", [P, G * O]) for i in range(3)]
        epT = [sb(f"epT{eh}", [P, O], F32) for eh in range(2)]
        epLD = [sb(f"epLD{eh}", [P, O], F32) for eh in range(2)]
        epLU = [sb(f"epLU{eh}", [P, O], F32) for eh in range(2)]
        epS = [sb(f"epS{eh}", [P, O], F32) for eh in range(2)]
        ob = [sb(f"ob{eh}", [P, O], F32) for eh in range(2)]
        tapb = sb("tapb", [P, O], F32)

        prj_ps = [ps(f"prj_ps{i}", [P, O]) for i in range(2)]
        D_ps = [ps(f"D_ps{eh}", [P, O]) for eh in range(2)]
        N_ps = [ps(f"N_ps{eh}", [P, O]) for eh in range(2)]

        s_lA = sem("s_lA")  # blobA: qT + Wq
        s_lC = sem("s_lC")  # blobB: Wk + Wv
        s_lB = sem("s_lB")  # bias + pos_bias
        s_gp = sem("s_gp")
        s_prj = sem("s_prj")
        s_pd = sem("s_pd")
        s_lg = sem("s_lg")
        s_ex = sem("s_ex")
        s_ev = sem("s_ev")
        s_pe = sem("s_pe")
        s_ea = sem("s_ea")
        s_ed = sem("s_ed")
        s_epi = sem("s_epi")
        s_od = sem("s_od")
        s_at = sem("s_at")

        def k_sh(eh, d):
            """K window-shifted AP, 4B-aligned: even offsets from K, odd
            from the 1-shifted copy."""
            o = HALO + d
            if o % 2 == 0:
                return QKV["k", eh][:, o:o + O]
            return K1[eh][:, o - 1:o - 1 + O]

        def v_sh(eh, d):
            o = HALO + d
            if o % 2 == 0:
                return QKV["v", eh][:, o:o + O]
            return V1[eh][:, o - 1:o - 1 + O]

        # projection groups: (eh, t, (n0, nn))
        groups = [(eh, t, c) for eh in range(2) for t in "qkv"
                  for c in ((0, 512), (512, NH - 512))]

        with nc.Block() as block:

            @block.sync
            def _(sync):
                # 4 consolidated loads: per-partition rows are contiguous
                # in DRAM so each DMA moves large packets. Separate
                # semaphores (HWDGE queues complete out of order).
                sync.dma_start(out=blobA, in_=blobA_d[:, :]
                               ).then_inc(s_lA, 16)
                sync.dma_start(out=blobB, in_=blobB_d[:, :]
                               ).then_inc(s_lC, 16)
                for eh in range(2):
                    sync.dma_start(out=fblob[eh], in_=fblob_d[eh][:, :]
                                   ).then_inc(s_lB, 16)
                if _DEBUG_TAP is None:
                    for eh in range(2):
                        sync.wait_ge(s_epi, eh + 1)
                        sync.dma_start(out=out_d[eh * P:(eh + 1) * P, :],
                                       in_=ob[eh]).then_inc(s_od, 16)
                    sync.wait_ge(s_od, 32)
                else:
                    sync.wait_ge(s_epi, 2)
                    tap = {
                        "D0": lambda: tapb,
                        "N0": lambda: tapb,
                        "out0": lambda: ob[0],
                    }[_DEBUG_TAP]()
                    tw = tap.shape[1]
                    sync.dma_start(out=out_d[0:P, 0:tw], in_=tap
                                   ).then_inc(s_od, 16)
                    sync.wait_ge(s_od, 16)

            @block.gpsimd
            def _(gpsimd):
                gpsimd.memset(ident, 0.0)
                gpsimd.affine_select(
                    out=ident, in_=ident,
                    compare_op=mybir.AluOpType.not_equal,
                    fill=1.0, base=0, pattern=[[-1, P]], channel_multiplier=1,
                ).then_inc(s_gp, 1)
                gpsimd.memset(ones, 1.0)
                gpsimd.memset(ones[:, 0:HALO], 0.0).then_inc(s_gp, 1)


            @block.tensor
            def _(tensor):
                tensor.wait_ge(s_gp, 2)
                # projections: ping-pong over two PSUM banks
                tensor.wait_ge(s_lA, 16)
                kv_waited = False
                for g, (eh, t, (n0, nn)) in enumerate(groups):
                    bank = prj_ps[g % 2]
                    if t != "q" and not kv_waited:
                        kv_waited = True
                        tensor.wait_ge(s_lC, 16)
                    if g >= 2:
                        dep = g - 2
                        if dep <= 3:
                            tensor.wait_ge(s_pd, dep + 1)
                        else:
                            tensor.wait_ge(s_prj, ACT_CUM[dep])
                    for kh in range(2):
                        tensor.matmul(
                            bank[:, :nn],
                            wT[t, kh][:, eh * P:(eh + 1) * P],
                            qT[kh][:, n0:n0 + nn],
                            start=(kh == 0), stop=(kh == 1),
                        ).then_inc(s_pe, 1)
                # window accumulation, one block of G offsets at a time
                for b in range(NBLK):
                    eh, d0 = blk_iters(b)
                    if b % 9 == 0:
                        V = QKV["v", eh]
                        tensor.wait_ge(s_prj, 3 if eh == 0 else 10)
                        tensor.matmul(D_ps[eh], ident, ones[:, HALO:HALO + O],
                                      start=True, stop=False).then_inc(s_pe, 1)
                        tensor.matmul(D_ps[eh], ident, ones[:, 0:O],
                                      start=False, stop=False).then_inc(s_pe, 1)
                        tensor.matmul(N_ps[eh], ident, V[:, 0:O],
                                      start=True, stop=False).then_inc(s_pe, 1)
                        tensor.matmul(N_ps[eh], ident,
                                      V[:, 2 * HALO:2 * HALO + O],
                                      start=False, stop=False).then_inc(s_pe, 1)
                    tensor.wait_ge(s_ev, 2 * (b + 1))
                    for g in range(G):
                        d = d0 + g
                        last = d == S - 1
                        tensor.matmul(N_ps[eh], ident,
                                      vbig[b % 3][:, g * O:(g + 1) * O],
                                      start=False, stop=last).then_inc(s_pe, 1)
                        vs = max(0, -d)
                        tensor.matmul(D_ps[eh][:, vs:], ident,
                                      ebig[b % 4][:, g * O + vs:(g + 1) * O],
                                      start=False, stop=last).then_inc(s_pe, 1)

            @block.vector
            def _(vector):
                def slab_ap(t, par, cnt, col0=0):
                    # every-other-slab view: [P, cnt, O] starting at slab par
                    return bass.AP(
                        tensor=t.tensor, offset=t.offset + (par * O) + col0,
                        ap=[t.ap[0], [2 * O, cnt], [1, O]])

                def emit_ev_block(bb):
                    ehb, d0b = blk_iters(bb)
                    vector.wait_ge(s_ex, bb + 1)
                    if bb >= 3:
                        vector.wait_ge(s_pe, pe_after_block(bb - 3))
                    for par in range(2):
                        cnt = (G + 1 - par) // 2
                        base = HALO + d0b + par
                        if base % 2 == 0:
                            vsrc, vo = QKV["v", ehb], base
                        else:
                            vsrc, vo = V1[ehb], base - 1
                        in1 = bass.AP(
                            tensor=vsrc.tensor, offset=vsrc.offset + vo,
                            ap=[vsrc.ap[0], [2, cnt], [1, O]])
                        vector.tensor_tensor(
                            out=slab_ap(vbig[bb % 3], par, cnt),
                            in0=slab_ap(ebig[bb % 4], par, cnt),
                            in1=in1, op=mult,
                        ).then_inc(s_ev, 1)

                vector.wait_ge(s_lB, 32)  # bias scalars present
                for g in range(4):
                    ehg, tg, (n0, nn) = groups[g]
                    ti = "qkv".index(tg)
                    vector.wait_ge(s_pe, 2 * (g + 1))
                    vector.tensor_scalar_add(
                        QKV[tg, ehg][:, n0:n0 + nn], prj_ps[g % 2][:, :nn],
                        bs[ehg][:, ti:ti + 1]).then_inc(s_pd, 1)
                for b in range(NBLK):
                    eh, d0 = blk_iters(b)
                    Q = QKV["q", eh]
                    if b % 9 == 0:
                        vector.wait_ge(s_prj, 3 if eh == 0 else 10)
                        vector.tensor_copy(K1[eh][:, 0:NH - 1],
                                           QKV["k", eh][:, 1:NH])
                        vector.tensor_copy(V1[eh][:, 0:NH - 1],
                                           QKV["v", eh][:, 1:NH])
                    if b < 2:
                        ndve = G
                    else:
                        ndve = G - 3 if b % 2 == 0 else G - 2
                    for g in range(ndve):
                        d = d0 + g
                        vector.tensor_scalar_add(
                            tbig[b % 2][:, g * O:(g + 1) * O], k_sh(eh, d),
                            pb[eh][:, d + S:d + S + 1])
                    if b >= 2:
                        vector.wait_ge(s_at, b - 1)  # ACT-side K+pb slabs
                    if b >= 4:
                        vector.wait_ge(s_ex, b - 3)  # lbig slot free
                    qb = bass.AP(
                        tensor=Q.tensor, offset=Q.offset + HALO,
                        ap=[Q.ap[0], [0, G], [1, O]])
                    tb3 = bass.AP(
                        tensor=tbig[b % 2].tensor, offset=tbig[b % 2].offset,
                        ap=[tbig[b % 2].ap[0], [O, G], [1, O]])
                    lb3 = bass.AP(
                        tensor=lbig[b % 4].tensor, offset=lbig[b % 4].offset,
                        ap=[lbig[b % 4].ap[0], [O, G], [1, O]])
                    vector.tensor_tensor(out=lb3, in0=tb3, in1=qb, op=mult
                                         ).then_inc(s_lg, 1)
                    if b >= 2:
                        emit_ev_block(b - 2)
                    if b == 2:
                        for eh in range(2):
                            vector.wait_ge(s_ea, eh + 1)
                            vector.tensor_scalar_add(epS[eh], epT[eh], 1.0
                                                     ).then_inc(s_ed, 1)
                    if b == 13:
                        vector.wait_ge(s_ea, 5)
                        vector.scalar_tensor_tensor(
                            out=epT[0], in0=epLU[0], scalar=2.0,
                            in1=epLD[0], op0=mult, op1=add,
                        ).then_inc(s_ed, 1)
                    if b == 15:
                        vector.wait_ge(s_ea, 6)
                        vector.tensor_mul(ob[0], N_ps[0], epS[0]
                                          ).then_inc(s_epi, 1)
                for bb in (NBLK - 2, NBLK - 1):
                    emit_ev_block(bb)

                # tail: out = N * exp(-(ln D + 2 ln(1+exp(-Q))))
                if _DEBUG_TAP in ("D0", "N0"):
                    vector.wait_ge(s_pe, PE_TOTAL)
                    vector.tensor_copy(
                        tapb, D_ps[0] if _DEBUG_TAP == "D0" else N_ps[0])
                vector.wait_ge(s_ea, 7)
                vector.scalar_tensor_tensor(
                    out=epT[1], in0=epLU[1], scalar=2.0,
                    in1=epLD[1], op0=mult, op1=add,
                ).then_inc(s_ed, 1)
                vector.wait_ge(s_ea, 8)
                vector.tensor_mul(ob[1], N_ps[1], epS[1]
                                  ).then_inc(s_epi, 1)

            @block.scalar
            def _(scalar):
                # projections: add bias, move PSUM -> SBUF (groups 4-11)
                scalar.wait_ge(s_lB, 32)  # bias + pos_bias present
                for g in range(4, 12):
                    eh, t, (n0, nn) = groups[g]
                    ti = "qkv".index(t)
                    bank = prj_ps[g % 2]
                    scalar.wait_ge(s_pe, 2 * (g + 1))
                    T_sb = QKV[t, eh]
                    if t == "v" and n0 == 0:
                        scalar.activation(T_sb[:, 0:HALO], bank[:, 0:HALO],
                                          AF.Copy).then_inc(s_prj, 1)
                        scalar.activation(
                            T_sb[:, HALO:nn], bank[:, HALO:nn], AF.Identity,
                            bias=bs[eh][:, ti:ti + 1], scale=1.0,
                        ).then_inc(s_prj, 1)
                    else:
                        scalar.activation(
                            T_sb[:, n0:n0 + nn], bank[:, :nn], AF.Identity,
                            bias=bs[eh][:, ti:ti + 1], scale=1.0,
                        ).then_inc(s_prj, 1)
                scalar.wait_ge(s_pd, 2)  # eh0 Q from the DVE
                for eh in range(2):
                    scalar.activation(epT[eh], QKV["q", eh][:, HALO:HALO + O],
                                      AF.Exp, scale=-1.0).then_inc(s_ea, 1)
                for b in range(NBLK):
                    ehb, d0b = blk_iters(b)
                    if b >= 2:
                        if b == 2:
                            scalar.wait_ge(s_pd, 4)  # eh0 K from the DVE
                        scalar.wait_ge(s_lg, b - 1)  # tbig slot free
                        gset = (G - 3, G - 2, G - 1) if b % 2 == 0 \
                            else (G - 2, G - 1)
                        for g in gset:
                            d = d0b + g
                            o = HALO + d
                            ai = scalar.activation(
                                tbig[b % 2][:, g * O:(g + 1) * O],
                                QKV["k", ehb][:, o:o + O], AF.Identity,
                                bias=pb[ehb][:, d + S:d + S + 1], scale=1.0)
                        ai.then_inc(s_at, 1)
                    if b >= 1:
                        bb = b - 1
                        scalar.wait_ge(s_lg, bb + 1)
                        if bb >= 4:
                            scalar.wait_ge(s_pe, pe_after_block(bb - 4))
                        scalar.activation(ebig[bb % 4], lbig[bb % 4], AF.Exp
                                          ).then_inc(s_ex, 1)
                        if bb == 4:
                            # LU = ln(1+exp(-Q)), both eh (u ready early)
                            for eh in range(2):
                                scalar.wait_ge(s_ed, eh + 1)
                                scalar.activation(epLU[eh], epS[eh], AF.Ln
                                                  ).then_inc(s_ea, 1)
                        if bb == 10:
                            scalar.wait_ge(s_pe, pe_after_block(8))
                            scalar.activation(epLD[0], D_ps[0], AF.Ln
                                              ).then_inc(s_ea, 1)
                        if bb == 12:
                            scalar.wait_ge(s_ed, 3)
                            scalar.activation(epS[0], epT[0], AF.Exp,
                                              scale=-1.0).then_inc(s_ea, 1)
                bb = NBLK - 1
                scalar.wait_ge(s_lg, bb + 1)
                scalar.wait_ge(s_pe, pe_after_block(bb - 4))
                scalar.activation(ebig[bb % 4], lbig[bb % 4], AF.Exp
                                  ).then_inc(s_ex, 1)
                # epilogue, same exp/ln table set (no set switch).
                # sigma-side (T = exp(-Q), LU = ln(1+T)) runs early; only
                # the D-dependent part trails the PE.
                scalar.wait_ge(s_pe, PE_TOTAL)
                scalar.activation(epLD[1], D_ps[1], AF.Ln).then_inc(s_ea, 1)
                scalar.wait_ge(s_ed, 4)
                scalar.activation(epS[1], epT[1], AF.Exp, scale=-1.0
                                  ).then_inc(s_ea, 1)

    return nc


def _shard_inputs(q, Wq, bq, Wk, bk, Wv, bv, pos_bias):
    """Build per-core input maps. Core c = 2*b + h."""
    wqT = Wq.T.astype(NPBF)
    wkT = Wk.T.astype(NPBF)
    wvT = Wv.T.astype(NPBF)
    bias = np.stack([bq, bk, bv], axis=1).astype(np.float32)   # [E, 3]
    pbT_f = pos_bias.T.astype(np.float32)                      # [E, W]
    pbT_r = pos_bias[::-1].T.astype(np.float32)                # reversed

    blobB = np.ascontiguousarray(np.concatenate(
        [wkT[0:P], wvT[0:P], wkT[P:E], wvT[P:E]], axis=1))     # [P, 4E]
    in_maps = []
    for c in range(8):
        b, h = divmod(c, 2)
        qh = np.zeros((NH, E), np.float32)
        if h == 0:
            qh[HALO:] = q[b, 0:O + HALO]          # positions -32..543, pad<0
        else:
            qh[HALO:] = q[b, L - (O + HALO):][::-1]  # reversed right half
        qT = qh.T.astype(NPBF)                                 # [E, NH]
        pbT = pbT_f if h == 0 else pbT_r
        fb = np.concatenate([pbT, bias], axis=1)               # [E, W+3]
        in_maps.append({
            "blobA": np.ascontiguousarray(np.concatenate(
                [qT[0:P], wqT[0:P], qT[P:E], wqT[P:E]], axis=1)),
            "blobB": blobB,
            "fblob0": np.ascontiguousarray(fb[0:P]),
            "fblob1": np.ascontiguousarray(fb[P:E]),
        })
    return in_maps


def _unshard(results):
    out = np.empty((B, L, E), np.float32)
    for c in range(8):
        b, h = divmod(c, 2)
        o_core = np.asarray(results[c]["out"], np.float32).T  # [O, E]
        if h == 0:
            out[b, 0:O] = o_core
        else:
            out[b, L - O:] = o_core[::-1]
    return out


def kernel(q, Wq, bq, Wk, bk, Wv, bv, pos_bias):
    global LAST_RESULTS
    q = np.asarray(q, np.float32)
    if "nc" not in _CACHE:
        _CACHE["nc"] = _build_nc()
    nc = _CACHE["nc"]
    in_maps = _shard_inputs(q, np.asarray(Wq), np.asarray(bq), np.asarray(Wk),
                            np.asarray(bk), np.asarray(Wv), np.asarray(bv),
                            np.asarray(pos_bias))
    res = bass_utils.run_bass_kernel_spmd(
        nc, in_maps, core_ids=list(range(8)), trace=TRACE,
    )
    LAST_RESULTS = res
    return _unshard(res.results)
